# revision 23
# baseline (speedup 1.0000x reference)
"""Trainium2 Bass kernel for nn_AttentionNet (pooling / ridge regime).

Model (per batch b of B=128, L=512, D=300, H=200, V=50000):
  word_emb = emb_table[words]                          [B,L,D]
  subj_emb = max over l with subj_pos[b,l]==0 of word_emb (else -1e12)
  obj_emb  = same with obj_pos
  hid  = tanh(word_emb @ w1[:D] + subj_emb @ w1[D:] + b1)
  attn = softmax(hid @ w2, axis=l)    (b2 dropped: softmax shift-invariant)
  subj_attn = sum_l attn * word_emb   (obj_attn identical -- source bug)
  out = relu(relu(cat([subj_attn, subj_attn, subj_emb, obj_emb]) @ mw1 + mb1) @ mw2 + mb2)

Sharding: pure data parallel, 16 batches per core on 8 cores; embedding
table and the small weights replicated.

All embeddings/weights are bf16 on device; PSUM accumulation, softmax
stats, biases and the final output stay fp32.

Device plan per core (16 batches = 16 token-tiles of 512):
  - all small parameters ship as 3 packed blobs (bf16 weights / f32
    biases / int16 indices) so startup is 3 DMAs, not ~40.
  - bulk gather via gpsimd.dma_gather (int16 indices, 1024 per call --
    larger prep descriptors hit a GPSIMD cost cliff).  The table is
    stored with one sentinel row (-1e12) at physical row 0 and one at
    row V+1 (word w -> physical row w+1).  The vocabulary exceeds int16
    range, so each batch's tokens are sorted by word id (attention +
    pools are order-invariant within a batch) and split into the 256
    smallest / 256 largest; low halves gather from table[0:32768], high
    halves from table[VP-32768:VP].
  - masked max-pools via a second gather: for each (batch, mask) the
    host emits the ~64 unmasked token ids padded with sentinel rows to
    64 lo + 64 hi slots.  Subject slots ship in the first pool calls so
    the tanh bias (w1b^T subj_emb + b1) unblocks before the main GEMM
    needs it; object slots only gate the final MLP.  On device: one
    slotwise lo/hi max, 3 PE transposes per 128-slot subtile, one
    segmented reduce -> both pools with zero mask arithmetic.
  - attention scores via bf16 matmuls on D-major PE-transposed
    embeddings; softmax + the attention-weighted sum run per 8-batch
    half, interleaved with the remaining GEMM stream to kill the
    serial tail.
  - 2-layer output MLP with the duplicated subj_attn block pre-folded
    into mw1 on the host (rows 0:300 += rows 300:600).
"""

import numpy as np

import concourse.bass as bass
import concourse.bacc as bacc
import concourse.mybir as mybir
import concourse.tile as tile
from concourse.masks import make_identity
from contextlib import ExitStack

F32 = mybir.dt.float32
BF16 = mybir.dt.bfloat16
I16 = mybir.dt.int16

NEG_INF = 1e12      # reference constant

# ---------------------------------------------------------------- config


class Cfg:
    def __init__(self, B=128, L=512, D=300, H=200, V=50000, NCORES=8,
                 PT=128, CW=128, HCW=100, gather_split=4, PSLOT=64):
        self.B, self.L, self.D, self.H, self.V = B, L, D, H, V
        self.NCORES = NCORES
        self.BC = B // NCORES          # batches per core
        self.PT = PT                   # token subtile (partitions)
        self.NSUB = L // PT            # subtiles per batch (must be even)
        self.NS = self.BC * self.NSUB  # token subtiles per core
        self.T = self.BC * L           # tokens per core
        self.CW = CW                   # D-chunk width
        self.HCW = HCW                 # H-chunk width
        self.gather_split = gather_split
        assert L % PT == 0 and H % HCW == 0 and self.NSUB % 2 == 0
        # gather element size in bf16 elements: row bytes padded to 256B
        self.E = -(-D * 2 // 256) * 128          # 384 for D=300
        # sentinel-augmented table: word w -> physical row w + 1
        self.VP = V + 2
        self.HB2 = self.VP - 32768     # high window start (17234)
        # pool slots per (batch, mask) per side
        self.PSLOT = PSLOT
        self.NPOOL = 2 * self.BC * PSLOT   # pool rows per side per core
        self.NPS = self.NPOOL // self.PT   # pool subtiles per side (16)
        # exact chunks of D (last may be narrow)
        self.dch = []
        s = 0
        while s < D:
            self.dch.append((s, min(CW, D - s)))
            s += CW
        self.hch = [(i * HCW, HCW) for i in range(H // HCW)]
        self.nd = len(self.dch)
        self.nh = len(self.hch)
        assert self.nd * CW == self.E  # bf16 rows tile exactly into chunks
        # packed weight blob column offsets (bf16, [128 | 100 rows])
        self.OW1A = 0
        self.OW1B = self.nd * self.H          # 600
        self.OMW1 = 2 * self.nd * self.H      # 1200
        self.OMW2 = self.OMW1 + 3 * self.nd * self.H   # 3000
        self.OW2 = self.OMW2 + self.nh * self.H        # 3400
        self.WBLOB = self.OW2 + self.nh                # 3402
        # idx blob column offsets (int16)
        self.OIXL = 0
        self.OIXH = (self.T // 2) // 16       # 256
        self.OIXPL = 2 * self.OIXH            # 512
        self.OIXPH = self.OIXPL + self.NPOOL // 16     # 640
        self.IBLOB = self.OIXPH + self.NPOOL // 16     # 768

    def subtiles(self, b):
        """Global subtile ids of batch b: low half then high half."""
        h = self.NSUB // 2
        lo = [h * b + k for k in range(h)]
        hi = [self.NS // 2 + h * b + k for k in range(h)]
        return lo + hi


# ------------------------------------------------------------- device IR


def build_nc(cfg: Cfg, queue_map=None):
    c = cfg
    nc = bacc.Bacc(num_swdge_queues=4)

    iblob_d = nc.declare_dram_parameter("iblob", [128, c.IBLOB], I16, isOutput=False)
    wblob_d = nc.declare_dram_parameter("wblob", [128, c.WBLOB], BF16, isOutput=False)
    bblob_d = nc.declare_dram_parameter("bblob", [c.HCW, 3 * c.nh], F32, isOutput=False)
    table = nc.declare_dram_parameter("table", [c.VP, c.E], BF16, isOutput=False)
    out_d = nc.declare_dram_parameter("out", [c.nh, c.HCW, c.BC], F32, isOutput=True)

    with tile.TileContext(nc) as tc, ExitStack() as ctx:
        sb = ctx.enter_context(tc.tile_pool(name="sb", bufs=1))

        # ---- persistent SBUF tiles
        iblob = sb.tile([128, c.IBLOB], I16)
        wblob = sb.tile([128, c.WBLOB], BF16)
        bblob = sb.tile([c.HCW, 3 * c.nh], F32)
        emb_tok = sb.tile([c.PT, c.NS, c.E], BF16)
        pool_lo = sb.tile([c.PT, c.NPS, c.E], BF16)
        pool_hi = sb.tile([c.PT, c.NPS, c.E], BF16)
        pooled = sb.tile([c.CW, c.nd, 2 * c.BC], BF16)   # [dlow, chunk, m*BC+b]
        ident = sb.tile([c.PT, c.PT], BF16)
        bias_sb = sb.tile([c.HCW, c.nh, c.BC], F32)
        NBH = c.BC // 2
        scores_h = [sb.tile([NBH, c.L], F32, name=f"scores{i}") for i in range(2)]
        attn_h = [sb.tile([NBH, c.L], F32, name=f"attn{i}") for i in range(2)]
        attn_bf = sb.tile([NBH, c.L], BF16)
        attn_t = sb.tile([c.PT, c.NSUB, c.BC], BF16)
        sattn = sb.tile([c.BC, c.D], F32)
        sattn_bf = sb.tile([c.BC, c.D], BF16)
        satd = sb.tile([c.CW, c.nd, c.BC], BF16)
        smax = [sb.tile([NBH, 1], F32, name=f"smax{i}") for i in range(2)]
        nsmax = [sb.tile([NBH, 1], F32, name=f"nsmax{i}") for i in range(2)]
        ssum = [sb.tile([NBH, 1], F32, name=f"ssum{i}") for i in range(2)]
        srec = [sb.tile([NBH, 1], F32, name=f"srec{i}") for i in range(2)]
        o1_sb = sb.tile([c.HCW, c.nh, c.BC], BF16)
        out_sb = sb.tile([c.HCW, c.nh, c.BC], F32)

        # weight blob views (flat column offsets)
        def w1a_T(ci, h0, hn, dn):
            return wblob[0:dn, c.OW1A + ci * c.H + h0:c.OW1A + ci * c.H + h0 + hn]

        def w1b_T(ci, h0, hn, dn):
            return wblob[0:dn, c.OW1B + ci * c.H + h0:c.OW1B + ci * c.H + h0 + hn]

        def mw1_T(k, h0, hn, dn):
            return wblob[0:dn, c.OMW1 + k * c.H + h0:c.OMW1 + k * c.H + h0 + hn]

        def mw2_T(ki, h0, hn, kn):
            return wblob[0:kn, c.OMW2 + ki * c.H + h0:c.OMW2 + ki * c.H + h0 + hn]

        def w2_T(hi, hn):
            return wblob[0:hn, c.OW2 + hi:c.OW2 + hi + 1]

        def bias_ap(kind, hi, hn):
            off = {"b1": 0, "mb1": c.nh, "mb2": 2 * c.nh}[kind] + hi
            return bblob[0:hn, off:off + 1]

        # ---- load blobs
        nc.sync.dma_start(out=iblob[:], in_=iblob_d[:])
        nc.sync.dma_start(out=wblob[:], in_=wblob_d[:])
        nc.sync.dma_start(out=bblob[:], in_=bblob_d[:])
        make_identity(nc, ident[:])

        # ---- gathers (all 1024-idx calls).  Order: subj pools + main k0
        # first, obj pools + k1, then k2, k3.  queue_map fixes each call's
        # queue to its scheduled DMASW lane (see find_queue_map).
        NHALF = c.T // 2
        nsp = c.gather_split
        npc = NHALF // nsp
        npp = c.NPOOL // 2
        assert npc % 128 == 0 and npp % 128 == 0
        qm = queue_map or {}
        calls = []

        def gather(name, out_ap, in_ap, i0, n, dflt_q):
            calls.append(name)
            nc.gpsimd.dma_gather(
                out_ap=out_ap, in_ap=in_ap, idxs_ap=iblob[:, i0:i0 + n // 16],
                num_idxs=n, num_idxs_reg=n, elem_size=c.E, single_packet=False,
                queue_num=qm.get(name, dflt_q))

        lo_win = table[0:32768, :]
        hi_win = table[c.HB2:c.VP, :]

        def main_pair(k, q0, q1):
            i0, s0 = k * (npc // 16), k * (npc // 128)
            gather(f"mlo{k}", emb_tok[:, s0:s0 + npc // 128, :], lo_win,
                   c.OIXL + i0, npc, q0)
            gather(f"mhi{k}", emb_tok[:, c.NS // 2 + s0:c.NS // 2 + s0 + npc // 128, :],
                   hi_win, c.OIXH + i0, npc, q1)

        def pool_pair(j, q0, q1):
            i0, s0 = j * (npp // 16), j * (npp // 128)
            gather(f"plo{j}", pool_lo[:, s0:s0 + npp // 128, :], lo_win,
                   c.OIXPL + i0, npp, q0)
            gather(f"phi{j}", pool_hi[:, s0:s0 + npp // 128, :], hi_win,
                   c.OIXPH + i0, npp, q1)

        pool_pair(0, 0, 1)      # subj slots
        main_pair(0, 2, 3)
        pool_pair(1, 0, 1)      # obj slots
        main_pair(1, 2, 3)
        main_pair(2, 0, 1)
        main_pair(3, 2, 3)

        # ---- pools: slotwise lo/hi max, transpose, segmented reduce, clamp.
        def pool_half(h, ppool):
            st0, st1 = h * (c.NPS // 2), (h + 1) * (c.NPS // 2)
            nc.vector.tensor_tensor(
                out=pool_lo[:, st0:st1, :], in0=pool_lo[:, st0:st1, :],
                in1=pool_hi[:, st0:st1, :], op=mybir.AluOpType.max)
            for st in range(st0, st1):
                pp = ppool.tile([c.CW, c.nd, c.PT], BF16, tag="pp")
                for ci in range(c.nd):
                    nc.tensor.transpose(
                        out=pp[:, ci, :],
                        in_=pool_lo[:, st, ci * c.CW:(ci + 1) * c.CW],
                        identity=ident[:])
                nc.vector.tensor_reduce(
                    out=pooled[:, :, 2 * st:2 * st + 2],
                    in_=pp[:].rearrange("p c (q s) -> p c q s", s=c.PSLOT),
                    axis=mybir.AxisListType.X, op=mybir.AluOpType.max)
            nc.vector.tensor_scalar_max(
                out=pooled[:, :, 2 * st0:2 * st1],
                in0=pooled[:, :, 2 * st0:2 * st1], scalar1=-NEG_INF)

        def pool_rhs(m, ci, bsl=None):
            dn = c.dch[ci][1]
            if bsl is None:
                bsl = slice(0, c.BC)
            return pooled[0:dn, ci, m * c.BC + bsl.start:m * c.BC + bsl.stop]

        GB = min(4, c.BC)
        NG = c.BC // GB

        def run_group(g, tpool, epool, hpool, spool, srpool, hspool):
            embds = []
            for b in range(g * GB, (g + 1) * GB):
                subs = c.subtiles(b)
                pt = tpool.tile([c.CW, c.nd, c.L], BF16, tag="pt")
                for ci in range(c.nd):
                    for si, s in enumerate(subs):
                        nc.tensor.transpose(
                            out=pt[:, ci, si * c.PT:(si + 1) * c.PT],
                            in_=emb_tok[:, s, ci * c.CW:(ci + 1) * c.CW],
                            identity=ident[:])
                emb_d = epool.tile([c.CW, c.nd, c.L], BF16, tag="embd")
                nc.scalar.copy(out=emb_d[:, 0, :], in_=pt[:, 0, :])
                nc.vector.tensor_copy(out=emb_d[:, 1:c.nd, :], in_=pt[:, 1:c.nd, :])
                embds.append(emb_d)
            for bi, b in enumerate(range(g * GB, (g + 1) * GB)):
                emb_d = embds[bi]
                hid = hspool.tile([c.HCW, c.nh, c.L], BF16, tag="hid")
                for hi, (h0, hn) in enumerate(c.hch):
                    ph = hpool.tile([c.HCW, c.L], F32, tag="ph")
                    for ci, (d0, dn) in enumerate(c.dch):
                        nc.tensor.matmul(
                            out=ph[0:hn, :],
                            lhsT=w1a_T(ci, h0, hn, dn),
                            rhs=emb_d[0:dn, ci, :],
                            start=(ci == 0), stop=(ci == c.nd - 1))
                    nc.scalar.activation(
                        out=hid[0:hn, hi, :], in_=ph[0:hn, :],
                        func=mybir.ActivationFunctionType.Tanh,
                        bias=bias_sb[0:hn, hi, b:b + 1], scale=1.0)
                ps = spool.tile([1, c.L], F32, tag="ps")
                for hi, (h0, hn) in enumerate(c.hch):
                    nc.tensor.matmul(
                        out=ps[:], lhsT=w2_T(hi, hn),
                        rhs=hid[0:hn, hi, :],
                        start=(hi == 0), stop=(hi == c.nh - 1))
                srow = srpool.tile([1, c.L], F32, tag="srow")
                nc.vector.tensor_copy(out=srow[:], in_=ps[:])
                nc.sync.dma_start(
                    out=scores_h[b // (c.BC // 2)][b % (c.BC // 2):b % (c.BC // 2) + 1, :],
                    in_=srow[:])

        def attn_half(h, apool, wpool, wrpool):
            # softmax + attention-weighted sum for batches [h*8, h*8+8)
            nb = c.BC // 2
            hsl = slice(h * nb, (h + 1) * nb)
            sc, at = scores_h[h], attn_h[h]
            nc.vector.tensor_reduce(out=smax[h][:], in_=sc[:],
                                    axis=mybir.AxisListType.X,
                                    op=mybir.AluOpType.max)
            nc.vector.tensor_scalar_mul(out=nsmax[h][:], in0=smax[h][:],
                                        scalar1=-1.0)
            nc.scalar.activation(out=at[:], in_=sc[:],
                                 func=mybir.ActivationFunctionType.Exp,
                                 bias=nsmax[h][:, 0:1], scale=1.0)
            nc.vector.tensor_reduce(out=ssum[h][:], in_=at[:],
                                    axis=mybir.AxisListType.X,
                                    op=mybir.AluOpType.add)
            nc.vector.reciprocal(out=srec[h][:], in_=ssum[h][:])
            nc.vector.tensor_scalar(out=attn_bf[:], in0=at[:],
                                    scalar1=srec[h][:, 0:1], scalar2=None,
                                    op0=mybir.AluOpType.mult)
            for si in range(c.NSUB):
                pa = apool.tile([c.PT, nb], BF16, tag="pa")
                nc.tensor.transpose(out=pa[:],
                                    in_=attn_bf[:, si * c.PT:(si + 1) * c.PT],
                                    identity=ident[0:nb, 0:nb])
                nc.vector.tensor_copy(out=attn_t[:, si, hsl], in_=pa[:])
            for b in range(hsl.start, hsl.stop):
                subs = c.subtiles(b)
                pw = wpool.tile([1, c.D], F32, tag="pw")
                for si, s in enumerate(subs):
                    nc.tensor.matmul(
                        out=pw[:],
                        lhsT=attn_t[:, si, b:b + 1],
                        rhs=emb_tok[:, s, 0:c.D],
                        start=(si == 0), stop=(si == c.NSUB - 1))
                wrow = wrpool.tile([1, c.D], F32, tag="wrow")
                nc.scalar.copy(out=wrow[:], in_=pw[:])
                nc.sync.dma_start(out=sattn[b:b + 1, :], in_=wrow[:])

        with tc.tile_pool(name="tpool", bufs=1, space="PSUM") as tpool, \
             tc.tile_pool(name="hpool", bufs=2, space="PSUM") as hpool, \
             tc.tile_pool(name="spool", bufs=1, space="PSUM") as spool, \
             tc.tile_pool(name="epool", bufs=2) as epool, \
             tc.tile_pool(name="srpool", bufs=3) as srpool, \
             tc.tile_pool(name="wrpool", bufs=3) as wrpool, \
             tc.tile_pool(name="hspool", bufs=2) as hspool:
            with tc.tile_pool(name="ppool", bufs=1, space="PSUM") as ppool, \
                 tc.tile_pool(name="bpool", bufs=1, space="PSUM") as bpool:
                pool_half(0, ppool)
                # tanh bias for all batches: w1b^T subj_emb + b1
                for hi, (h0, hn) in enumerate(c.hch):
                    pb = bpool.tile([c.HCW, c.BC], F32, tag="pb")
                    for ci, (d0, dn) in enumerate(c.dch):
                        nc.tensor.matmul(
                            out=pb[0:hn, :],
                            lhsT=w1b_T(ci, h0, hn, dn),
                            rhs=pool_rhs(0, ci),
                            start=(ci == 0), stop=(ci == c.nd - 1))
                    nc.scalar.activation(
                        out=bias_sb[0:hn, hi, :], in_=pb[0:hn, :],
                        func=mybir.ActivationFunctionType.Identity,
                        bias=bias_ap("b1", hi, hn), scale=1.0)
                run_group(0, tpool, epool, hpool, spool, srpool, hspool)
                run_group(1, tpool, epool, hpool, spool, srpool, hspool)
                pool_half(1, ppool)     # obj pools; only gate the MLP
            with tc.tile_pool(name="apool", bufs=1, space="PSUM") as apool, \
                 tc.tile_pool(name="wpool", bufs=2, space="PSUM") as wpool:
                attn_half(0, apool, wpool, wrpool)
                run_group(2, tpool, epool, hpool, spool, srpool, hspool)
                run_group(3, tpool, epool, hpool, spool, srpool, hspool)
                attn_half(1, apool, wpool, wrpool)

        # ---- transpose sattn to D-major chunks [dlow, chunk, b]
        nc.vector.tensor_copy(out=sattn_bf[:], in_=sattn[:])
        with tc.tile_pool(name="stpool", bufs=2, space="PSUM") as stpool:
            for ci, (d0, dn) in enumerate(c.dch):
                pst = stpool.tile([c.CW, c.BC], BF16, tag="pst")
                nc.tensor.transpose(out=pst[0:dn, :], in_=sattn_bf[:, d0:d0 + dn],
                                    identity=ident[0:c.BC, 0:c.BC])
                nc.vector.tensor_copy(out=satd[0:dn, ci, :], in_=pst[0:dn, :])

        # ---- output MLP (N=BC is small)
        with tc.tile_pool(name="mpool", bufs=2, space="PSUM") as mpool, \
             tc.tile_pool(name="m2pool", bufs=2, space="PSUM") as m2pool:
            nk = 3 * c.nd
            for hi, (h0, hn) in enumerate(c.hch):
                pm = mpool.tile([c.HCW, c.BC], F32, tag="pm")
                for blk in range(3):
                    for ci, (d0, dn) in enumerate(c.dch):
                        k = blk * c.nd + ci
                        if blk == 0:
                            rhs = satd[0:dn, ci, :]
                        else:
                            rhs = pool_rhs(blk - 1, ci)
                        nc.tensor.matmul(
                            out=pm[0:hn, :],
                            lhsT=mw1_T(k, h0, hn, dn),
                            rhs=rhs, start=(k == 0), stop=(k == nk - 1))
                nc.scalar.activation(
                    out=o1_sb[0:hn, hi, :], in_=pm[0:hn, :],
                    func=mybir.ActivationFunctionType.Relu,
                    bias=bias_ap("mb1", hi, hn), scale=1.0)
            for hi, (h0, hn) in enumerate(c.hch):
                pm2 = m2pool.tile([c.HCW, c.BC], F32, tag="pm2")
                for ki, (k0, kn) in enumerate(c.hch):
                    nc.tensor.matmul(
                        out=pm2[0:hn, :],
                        lhsT=mw2_T(ki, h0, hn, kn),
                        rhs=o1_sb[0:kn, ki, :],
                        start=(ki == 0), stop=(ki == c.nh - 1))
                nc.scalar.activation(
                    out=out_sb[0:hn, hi, :], in_=pm2[0:hn, :],
                    func=mybir.ActivationFunctionType.Relu,
                    bias=bias_ap("mb2", hi, hn), scale=1.0)
            for hi in range(c.nh):
                nc.sync.dma_start(out=out_d[hi], in_=out_sb[:, hi, :])

    nc.finalize()
    nc._gather_call_names = calls
    return nc


def find_queue_map(cfg: Cfg):
    """Assign each gather call the queue implied by its scheduled DMASW
    lane (lane rotates per scheduled SWDGE DMA; a lane's semaphore is
    locked to one queue).  Iterate to a fixed point."""
    import bass_rust
    nsem = bass_rust.NUM_SWDGE_GLOBAL_SEMS
    qm = {}
    for _ in range(4):
        nc = build_nc(cfg, queue_map=qm)
        order = []
        for fn in nc.m.functions:
            for blk in fn.blocks:
                for inst in blk.instructions:
                    if 'Gather' in type(inst).__name__:
                        order.append((inst.name, getattr(inst, 'queue_num', 0)))
        emit = nc._gather_call_names
        by_emit = sorted((nm for nm, _ in order), key=lambda s: int(s.split('-')[1]))
        new_qm = {}
        ok = True
        lane_q = {}
        for sched_i, (nm, q) in enumerate(order):
            emit_idx = by_emit.index(nm)
            lane = sched_i % nsem
            want = lane_q.setdefault(lane, lane % 4)
            new_qm[emit[emit_idx]] = want
            if q != want:
                ok = False
        qm = new_qm
        if ok:
            return qm
    return qm


# ------------------------------------------------------------------ host


def wrap16(idx, n):
    """int16 index list -> [128, n/16] wrapped + replicated per Q7 core."""
    return np.ascontiguousarray(
        np.tile(np.asarray(idx).astype(np.int16).reshape(n // 16, 16).T, (8, 1)))


def to_bf16(x):
    import ml_dtypes
    return np.asarray(x, dtype=np.float32).astype(ml_dtypes.bfloat16)


def host_prepare(cfg: Cfg, words, subj_pos, obj_pos, emb_table,
                 w1, b1, w2, b2, mw1, mb1, mw2, mb2):
    import ml_dtypes
    c = cfg
    words = np.asarray(words).astype(np.int64)
    subj_pos = np.asarray(subj_pos)
    obj_pos = np.asarray(obj_pos)

    # sentinel rows at physical 0 and VP-1; word w -> row w + 1
    table = np.zeros((c.VP, c.E), ml_dtypes.bfloat16)
    table[1:1 + c.V, :c.D] = to_bf16(emb_table)
    table[0, :] = ml_dtypes.bfloat16(-NEG_INF)
    table[c.VP - 1, :] = ml_dtypes.bfloat16(-NEG_INF)

    w1 = np.asarray(w1, dtype=np.float32)
    w1a, w1b = w1[:c.D], w1[c.D:2 * c.D]
    mw1 = np.asarray(mw1, dtype=np.float32)
    mw1e = np.concatenate([mw1[0:c.D] + mw1[c.D:2 * c.D],
                           mw1[2 * c.D:3 * c.D], mw1[3 * c.D:4 * c.D]], axis=0)
    mw2 = np.asarray(mw2, dtype=np.float32)
    w2v = np.asarray(w2, dtype=np.float32).reshape(c.H)

    wblob = np.zeros((128, c.WBLOB), np.float32)
    for ci, (d0, dn) in enumerate(c.dch):
        wblob[0:dn, c.OW1A + ci * c.H:c.OW1A + (ci + 1) * c.H] = w1a[d0:d0 + dn]
        wblob[0:dn, c.OW1B + ci * c.H:c.OW1B + (ci + 1) * c.H] = w1b[d0:d0 + dn]
        for blk in range(3):
            k = blk * c.nd + ci
            wblob[0:dn, c.OMW1 + k * c.H:c.OMW1 + (k + 1) * c.H] = \
                mw1e[blk * c.D + d0:blk * c.D + d0 + dn]
    for ki, (k0, kn) in enumerate(c.hch):
        wblob[0:kn, c.OMW2 + ki * c.H:c.OMW2 + (ki + 1) * c.H] = mw2[k0:k0 + kn]
        wblob[0:kn, c.OW2 + ki] = w2v[k0:k0 + kn]

    bblob = np.zeros((c.HCW, 3 * c.nh), np.float32)
    for hi, (h0, hn) in enumerate(c.hch):
        bblob[0:hn, hi] = np.asarray(b1, np.float32).reshape(-1)[h0:h0 + hn]
        bblob[0:hn, c.nh + hi] = np.asarray(mb1, np.float32).reshape(-1)[h0:h0 + hn]
        bblob[0:hn, 2 * c.nh + hi] = np.asarray(mb2, np.float32).reshape(-1)[h0:h0 + hn]

    shared = {"table": table, "wblob": to_bf16(wblob),
              "bblob": np.ascontiguousarray(bblob)}
    HALF = c.L // 2
    in_maps = []
    for core in range(c.NCORES):
        b0 = core * c.BC
        lo_list, hi_list = [], []
        # pool slot arrays, ordered q = m*BC + b
        pl = np.zeros((2 * c.BC, c.PSLOT), np.int64)          # sentinel row 0
        ph = np.full((2 * c.BC, c.PSLOT), 32767, np.int64)    # sentinel VP-1
        for b in range(c.BC):
            w = words[b0 + b]
            order = np.argsort(w, kind="stable")
            ws = w[order] + 1                       # physical rows
            if ws[HALF - 1] >= 32768 or ws[HALF] < c.HB2:
                raise RuntimeError(
                    f"batch {b0 + b}: vocab split infeasible "
                    f"(lo_max={ws[HALF - 1]}, hi_min={ws[HALF]})")
            lo_list.append(ws[:HALF])
            hi_list.append(ws[HALF:] - c.HB2)
            for m, pos in ((0, subj_pos), (1, obj_pos)):
                kept = w[np.asarray(pos[b0 + b]) == 0] + 1    # physical rows
                lo = [int(x) for x in kept[kept < c.HB2]]
                hi = [int(x) for x in kept[kept > 32767]]
                for x in kept[(kept >= c.HB2) & (kept <= 32767)]:
                    (lo if len(lo) < c.PSLOT else hi).append(int(x))
                if len(lo) > c.PSLOT or len(hi) > c.PSLOT:
                    raise RuntimeError(f"pool slot overflow b={b0+b} m={m}: "
                                       f"{len(lo)}/{len(hi)}")
                q = m * c.BC + b
                pl[q, :len(lo)] = lo
                ph[q, :len(hi)] = [x - c.HB2 for x in hi]
        iblob = np.concatenate([
            wrap16(np.concatenate(lo_list), c.T // 2),
            wrap16(np.concatenate(hi_list), c.T // 2),
            wrap16(pl.reshape(-1), c.NPOOL),
            wrap16(ph.reshape(-1), c.NPOOL)], axis=1)
        in_maps.append({"iblob": np.ascontiguousarray(iblob), **shared})
    return in_maps


def assemble_output(cfg: Cfg, results):
    c = cfg
    outs = []
    for core in range(c.NCORES):
        o = results[core]["out"]                      # [nh, HCW, BC]
        outs.append(o.reshape(c.H, c.BC).T)           # [BC, H]
    return np.ascontiguousarray(np.concatenate(outs, axis=0))


_CACHE = {}


def run(inputs, trace=False, **kw):
    from concourse.bass_utils import run_bass_kernel_spmd

    cfg = Cfg()
    in_maps = host_prepare(cfg, **{k: inputs[k] for k in (
        "words", "subj_pos", "obj_pos", "emb_table", "w1", "b1", "w2", "b2",
        "mw1", "mb1", "mw2", "mb2")})
    if "nc" not in _CACHE:
        qm = find_queue_map(cfg)
        _CACHE["nc"] = build_nc(cfg, queue_map=qm)
    nc = _CACHE["nc"]
    res = run_bass_kernel_spmd(nc, in_maps, core_ids=list(range(cfg.NCORES)),
                               trace=trace, **kw)
    return assemble_output(cfg, res.results), res


def kernel(**inputs) -> np.ndarray:
    return run(inputs)[0]


# revision 24
# speedup vs baseline: 1.2587x; 1.2587x over previous
"""Trainium2 Bass kernel for nn_AttentionNet (pooling / ridge regime).

Model (per batch b of B=128, L=512, D=300, H=200, V=50000):
  word_emb = emb_table[words]                          [B,L,D]
  subj_emb = max over l with subj_pos[b,l]==0 of word_emb (else -1e12)
  obj_emb  = same with obj_pos
  hid  = tanh(word_emb @ w1[:D] + subj_emb @ w1[D:] + b1)
  attn = softmax(hid @ w2, axis=l)    (b2 dropped: softmax shift-invariant)
  subj_attn = sum_l attn * word_emb   (obj_attn identical -- source bug)
  out = relu(relu(cat([subj_attn, subj_attn, subj_emb, obj_emb]) @ mw1 + mb1) @ mw2 + mb2)

Sharding: pure data parallel, 16 batches per core on 8 cores; embedding
table and the small weights replicated.

All embeddings/weights are bf16 on device; PSUM accumulation, softmax
stats, biases and the final output stay fp32.

Device plan per core (16 batches = 16 token-tiles of 512):
  - all small parameters ship as 3 packed blobs (bf16 weights / f32
    biases / int16 indices) so startup is 3 DMAs, not ~40.
  - bulk gather via gpsimd.dma_gather (int16 indices, 1024 per call --
    larger prep descriptors hit a GPSIMD cost cliff).  The table is
    stored with one sentinel row (-1e12) at physical row 0 and one at
    row V+1 (word w -> physical row w+1).  The vocabulary exceeds int16
    range, so each batch's tokens are sorted by word id (attention +
    pools are order-invariant within a batch) and split into the 256
    smallest / 256 largest; low halves gather from table[0:32768], high
    halves from table[VP-32768:VP].
  - masked max-pools via a second gather: for each (batch, mask) the
    host emits the ~64 unmasked token ids padded with sentinel rows to
    64 lo + 64 hi slots.  Subject slots ship in the first pool calls so
    the tanh bias (w1b^T subj_emb + b1) unblocks before the main GEMM
    needs it; object slots only gate the final MLP.  On device: one
    slotwise lo/hi max, 3 PE transposes per 128-slot subtile, one
    segmented reduce -> both pools with zero mask arithmetic.
  - attention scores via bf16 matmuls on D-major PE-transposed
    embeddings; softmax + the attention-weighted sum run per 8-batch
    half, interleaved with the remaining GEMM stream to kill the
    serial tail.
  - 2-layer output MLP with the duplicated subj_attn block pre-folded
    into mw1 on the host (rows 0:300 += rows 300:600).
"""

import numpy as np

import concourse.bass as bass
import concourse.bacc as bacc
import concourse.mybir as mybir
import concourse.tile as tile
from concourse.masks import make_identity
from contextlib import ExitStack

F32 = mybir.dt.float32
BF16 = mybir.dt.bfloat16
I16 = mybir.dt.int16

NEG_INF = 1e12      # reference constant

# ---------------------------------------------------------------- config


class Cfg:
    def __init__(self, B=128, L=512, D=300, H=200, V=50000, NCORES=8,
                 PT=128, CW=128, HCW=100, gather_split=4, PSLOT=64):
        self.B, self.L, self.D, self.H, self.V = B, L, D, H, V
        self.NCORES = NCORES
        self.BC = B // NCORES          # batches per core
        self.PT = PT                   # token subtile (partitions)
        self.NSUB = L // PT            # subtiles per batch (must be even)
        self.NS = self.BC * self.NSUB  # token subtiles per core
        self.T = self.BC * L           # tokens per core
        self.CW = CW                   # D-chunk width
        self.HCW = HCW                 # H-chunk width
        self.gather_split = gather_split
        assert L % PT == 0 and H % HCW == 0 and self.NSUB % 2 == 0
        # gather element size in bf16 elements: row bytes padded to 256B
        self.E = -(-D * 2 // 256) * 128          # 384 for D=300
        # sentinel-augmented table: word w -> physical row w + 1
        self.VP = V + 2
        self.HB2 = self.VP - 32768     # high window start (17234)
        # pool slots per (batch, mask) per side
        self.PSLOT = PSLOT
        self.NPOOL = 2 * self.BC * PSLOT   # pool rows per side per core
        self.NPS = self.NPOOL // self.PT   # pool subtiles per side (16)
        # exact chunks of D (last may be narrow)
        self.dch = []
        s = 0
        while s < D:
            self.dch.append((s, min(CW, D - s)))
            s += CW
        self.hch = [(i * HCW, HCW) for i in range(H // HCW)]
        self.nd = len(self.dch)
        self.nh = len(self.hch)
        assert self.nd * CW == self.E  # bf16 rows tile exactly into chunks
        # packed weight blob column offsets (bf16, [128 | 100 rows])
        self.OW1A = 0
        self.OW1B = self.nd * self.H          # 600
        self.OMW1 = 2 * self.nd * self.H      # 1200
        self.OMW2 = self.OMW1 + 3 * self.nd * self.H   # 3000
        self.OW2 = self.OMW2 + self.nh * self.H        # 3400
        self.WBLOB = self.OW2 + self.nh                # 3402
        # idx blob column offsets (int16)
        self.OIXL = 0
        self.OIXH = (self.T // 2) // 16       # 256
        self.OIXPL = 2 * self.OIXH            # 512
        self.OIXPH = self.OIXPL + self.NPOOL // 16     # 640
        self.IBLOB = self.OIXPH + self.NPOOL // 16     # 768

    def subtiles(self, b):
        """Global subtile ids of batch b: low half then high half."""
        h = self.NSUB // 2
        lo = [h * b + k for k in range(h)]
        hi = [self.NS // 2 + h * b + k for k in range(h)]
        return lo + hi


# ------------------------------------------------------------- device IR


def build_nc(cfg: Cfg, queue_map=None):
    c = cfg
    nc = bacc.Bacc(num_swdge_queues=4)

    iblob_d = nc.declare_dram_parameter("iblob", [128, c.IBLOB], I16, isOutput=False)
    wblob_d = nc.declare_dram_parameter("wblob", [128, c.WBLOB], BF16, isOutput=False)
    bblob_d = nc.declare_dram_parameter("bblob", [c.HCW, 3 * c.nh], F32, isOutput=False)
    table = nc.declare_dram_parameter("table", [c.VP, c.E], BF16, isOutput=False)
    out_d = nc.declare_dram_parameter("out", [c.nh, c.HCW, c.BC], F32, isOutput=True)

    with tile.TileContext(nc) as tc, ExitStack() as ctx:
        sb = ctx.enter_context(tc.tile_pool(name="sb", bufs=1))

        # ---- persistent SBUF tiles
        iblob = sb.tile([128, c.IBLOB], I16)
        wblob = sb.tile([128, c.WBLOB], BF16)
        bblob = sb.tile([c.HCW, 3 * c.nh], F32)
        emb_tok = sb.tile([c.PT, c.NS, c.E], BF16)
        pool_lo = sb.tile([c.PT, c.NPS, c.E], BF16)
        pool_hi = sb.tile([c.PT, c.NPS, c.E], BF16)
        pooled = sb.tile([c.CW, c.nd, 2 * c.BC], BF16)   # [dlow, chunk, m*BC+b]
        ident = sb.tile([c.PT, c.PT], BF16)
        bias_sb = sb.tile([c.HCW, c.nh, c.BC], F32)
        NBH = c.BC // 2
        scores_h = [sb.tile([NBH, c.L], F32, name=f"scores{i}") for i in range(2)]
        attn_h = [sb.tile([NBH, c.L], F32, name=f"attn{i}") for i in range(2)]
        attn_bf = sb.tile([NBH, c.L], BF16)
        attn_t = sb.tile([c.PT, c.NSUB, c.BC], BF16)
        sattn = sb.tile([c.BC, c.D], F32)
        sattn_bf = sb.tile([c.BC, c.D], BF16)
        satd = sb.tile([c.CW, c.nd, c.BC], BF16)
        smax = [sb.tile([NBH, 1], F32, name=f"smax{i}") for i in range(2)]
        nsmax = [sb.tile([NBH, 1], F32, name=f"nsmax{i}") for i in range(2)]
        ssum = [sb.tile([NBH, 1], F32, name=f"ssum{i}") for i in range(2)]
        srec = [sb.tile([NBH, 1], F32, name=f"srec{i}") for i in range(2)]
        o1_sb = sb.tile([c.HCW, c.nh, c.BC], BF16)
        out_sb = sb.tile([c.HCW, c.nh, c.BC], F32)

        # weight blob views (flat column offsets)
        def w1a_T(ci, h0, hn, dn):
            return wblob[0:dn, c.OW1A + ci * c.H + h0:c.OW1A + ci * c.H + h0 + hn]

        def w1b_T(ci, h0, hn, dn):
            return wblob[0:dn, c.OW1B + ci * c.H + h0:c.OW1B + ci * c.H + h0 + hn]

        def mw1_T(k, h0, hn, dn):
            return wblob[0:dn, c.OMW1 + k * c.H + h0:c.OMW1 + k * c.H + h0 + hn]

        def mw2_T(ki, h0, hn, kn):
            return wblob[0:kn, c.OMW2 + ki * c.H + h0:c.OMW2 + ki * c.H + h0 + hn]

        def w2_T(hi, hn):
            return wblob[0:hn, c.OW2 + hi:c.OW2 + hi + 1]

        def bias_ap(kind, hi, hn):
            off = {"b1": 0, "mb1": c.nh, "mb2": 2 * c.nh}[kind] + hi
            return bblob[0:hn, off:off + 1]

        # ---- load blobs
        nc.sync.dma_start(out=iblob[:], in_=iblob_d[:])
        nc.sync.dma_start(out=wblob[:], in_=wblob_d[:])
        nc.sync.dma_start(out=bblob[:], in_=bblob_d[:])
        make_identity(nc, ident[:])

        # ---- gathers (all 1024-idx calls).  Order: subj pools + main k0
        # first, obj pools + k1, then k2, k3.  queue_map fixes each call's
        # queue to its scheduled DMASW lane (see find_queue_map).
        NHALF = c.T // 2
        nsp = c.gather_split
        npc = NHALF // nsp
        npp = c.NPOOL // 2
        assert npc % 128 == 0 and npp % 128 == 0
        qm = queue_map or {}
        calls = []

        def gather(name, out_ap, in_ap, i0, n, dflt_q):
            calls.append(name)
            nc.gpsimd.dma_gather(
                out_ap=out_ap, in_ap=in_ap, idxs_ap=iblob[:, i0:i0 + n // 16],
                num_idxs=n, num_idxs_reg=n, elem_size=c.E, single_packet=False,
                queue_num=qm.get(name, dflt_q))

        lo_win = table[0:32768, :]
        hi_win = table[c.HB2:c.VP, :]

        def main_pair(k, q0, q1):
            i0, s0 = k * (npc // 16), k * (npc // 128)
            gather(f"mlo{k}", emb_tok[:, s0:s0 + npc // 128, :], lo_win,
                   c.OIXL + i0, npc, q0)
            gather(f"mhi{k}", emb_tok[:, c.NS // 2 + s0:c.NS // 2 + s0 + npc // 128, :],
                   hi_win, c.OIXH + i0, npc, q1)

        def pool_pair(j, q0, q1):
            i0, s0 = j * (npp // 16), j * (npp // 128)
            gather(f"plo{j}", pool_lo[:, s0:s0 + npp // 128, :], lo_win,
                   c.OIXPL + i0, npp, q0)
            gather(f"phi{j}", pool_hi[:, s0:s0 + npp // 128, :], hi_win,
                   c.OIXPH + i0, npp, q1)

        pool_pair(0, 0, 1)      # subj slots
        main_pair(0, 2, 3)
        pool_pair(1, 0, 1)      # obj slots
        main_pair(1, 2, 3)
        main_pair(2, 0, 1)
        main_pair(3, 2, 3)

        # ---- pools: slotwise lo/hi max, transpose, segmented reduce, clamp.
        def pool_half(h, ppool):
            st0, st1 = h * (c.NPS // 2), (h + 1) * (c.NPS // 2)
            nc.vector.tensor_tensor(
                out=pool_lo[:, st0:st1, :], in0=pool_lo[:, st0:st1, :],
                in1=pool_hi[:, st0:st1, :], op=mybir.AluOpType.max)
            for st in range(st0, st1):
                pp = ppool.tile([c.CW, c.nd, c.PT], BF16, tag="pp")
                for ci in range(c.nd):
                    nc.tensor.transpose(
                        out=pp[:, ci, :],
                        in_=pool_lo[:, st, ci * c.CW:(ci + 1) * c.CW],
                        identity=ident[:])
                nc.vector.tensor_reduce(
                    out=pooled[:, :, 2 * st:2 * st + 2],
                    in_=pp[:].rearrange("p c (q s) -> p c q s", s=c.PSLOT),
                    axis=mybir.AxisListType.X, op=mybir.AluOpType.max)
            nc.vector.tensor_scalar_max(
                out=pooled[:, :, 2 * st0:2 * st1],
                in0=pooled[:, :, 2 * st0:2 * st1], scalar1=-NEG_INF)

        def pool_rhs(m, ci, bsl=None):
            dn = c.dch[ci][1]
            if bsl is None:
                bsl = slice(0, c.BC)
            return pooled[0:dn, ci, m * c.BC + bsl.start:m * c.BC + bsl.stop]

        GB = min(4, c.BC)
        NG = c.BC // GB

        def run_group(g, tpool, epool, hpool, spool, srpool, hspool):
            embds = []
            for b in range(g * GB, (g + 1) * GB):
                subs = c.subtiles(b)
                pt = tpool.tile([c.CW, c.nd, c.L], BF16, tag="pt")
                for ci in range(c.nd):
                    for si, s in enumerate(subs):
                        nc.tensor.transpose(
                            out=pt[:, ci, si * c.PT:(si + 1) * c.PT],
                            in_=emb_tok[:, s, ci * c.CW:(ci + 1) * c.CW],
                            identity=ident[:])
                emb_d = epool.tile([c.CW, c.nd, c.L], BF16, tag="embd")
                nc.scalar.copy(out=emb_d[:, 0, :], in_=pt[:, 0, :])
                nc.vector.tensor_copy(out=emb_d[:, 1:c.nd, :], in_=pt[:, 1:c.nd, :])
                embds.append(emb_d)
            for bi, b in enumerate(range(g * GB, (g + 1) * GB)):
                emb_d = embds[bi]
                hid = hspool.tile([c.HCW, c.nh, c.L], BF16, tag="hid")
                for hi, (h0, hn) in enumerate(c.hch):
                    ph = hpool.tile([c.HCW, c.L], F32, tag="ph")
                    for ci, (d0, dn) in enumerate(c.dch):
                        nc.tensor.matmul(
                            out=ph[0:hn, :],
                            lhsT=w1a_T(ci, h0, hn, dn),
                            rhs=emb_d[0:dn, ci, :],
                            start=(ci == 0), stop=(ci == c.nd - 1))
                    nc.scalar.activation(
                        out=hid[0:hn, hi, :], in_=ph[0:hn, :],
                        func=mybir.ActivationFunctionType.Tanh,
                        bias=bias_sb[0:hn, hi, b:b + 1], scale=1.0)
                ps = spool.tile([1, c.L], F32, tag="ps")
                for hi, (h0, hn) in enumerate(c.hch):
                    nc.tensor.matmul(
                        out=ps[:], lhsT=w2_T(hi, hn),
                        rhs=hid[0:hn, hi, :],
                        start=(hi == 0), stop=(hi == c.nh - 1))
                srow = srpool.tile([1, c.L], F32, tag="srow")
                nc.vector.tensor_copy(out=srow[:], in_=ps[:])
                nc.sync.dma_start(
                    out=scores_h[b // (c.BC // 2)][b % (c.BC // 2):b % (c.BC // 2) + 1, :],
                    in_=srow[:])

        def attn_half(h, apool, wpool, wrpool):
            # softmax + attention-weighted sum for batches [h*8, h*8+8)
            nb = c.BC // 2
            hsl = slice(h * nb, (h + 1) * nb)
            sc, at = scores_h[h], attn_h[h]
            nc.vector.tensor_reduce(out=smax[h][:], in_=sc[:],
                                    axis=mybir.AxisListType.X,
                                    op=mybir.AluOpType.max)
            nc.vector.tensor_scalar_mul(out=nsmax[h][:], in0=smax[h][:],
                                        scalar1=-1.0)
            nc.scalar.activation(out=at[:], in_=sc[:],
                                 func=mybir.ActivationFunctionType.Exp,
                                 bias=nsmax[h][:, 0:1], scale=1.0)
            nc.vector.tensor_reduce(out=ssum[h][:], in_=at[:],
                                    axis=mybir.AxisListType.X,
                                    op=mybir.AluOpType.add)
            nc.vector.reciprocal(out=srec[h][:], in_=ssum[h][:])
            nc.vector.tensor_scalar(out=attn_bf[:], in0=at[:],
                                    scalar1=srec[h][:, 0:1], scalar2=None,
                                    op0=mybir.AluOpType.mult)
            for si in range(c.NSUB):
                pa = apool.tile([c.PT, nb], BF16, tag="pa")
                nc.tensor.transpose(out=pa[:],
                                    in_=attn_bf[:, si * c.PT:(si + 1) * c.PT],
                                    identity=ident[0:nb, 0:nb])
                nc.vector.tensor_copy(out=attn_t[:, si, hsl], in_=pa[:])
            for b in range(hsl.start, hsl.stop):
                subs = c.subtiles(b)
                pw = wpool.tile([1, c.D], F32, tag="pw")
                for si, s in enumerate(subs):
                    nc.tensor.matmul(
                        out=pw[:],
                        lhsT=attn_t[:, si, b:b + 1],
                        rhs=emb_tok[:, s, 0:c.D],
                        start=(si == 0), stop=(si == c.NSUB - 1))
                wrow = wrpool.tile([1, c.D], F32, tag="wrow")
                nc.scalar.copy(out=wrow[:], in_=pw[:])
                nc.sync.dma_start(out=sattn[b:b + 1, :], in_=wrow[:])

        with tc.tile_pool(name="tpool", bufs=1, space="PSUM") as tpool, \
             tc.tile_pool(name="hpool", bufs=2, space="PSUM") as hpool, \
             tc.tile_pool(name="spool", bufs=1, space="PSUM") as spool, \
             tc.tile_pool(name="epool", bufs=2) as epool, \
             tc.tile_pool(name="srpool", bufs=3) as srpool, \
             tc.tile_pool(name="wrpool", bufs=3) as wrpool, \
             tc.tile_pool(name="hspool", bufs=2) as hspool:
            with tc.tile_pool(name="ppool", bufs=1, space="PSUM") as ppool, \
                 tc.tile_pool(name="bpool", bufs=1, space="PSUM") as bpool:
                pool_half(0, ppool)
                # tanh bias for all batches: w1b^T subj_emb + b1
                for hi, (h0, hn) in enumerate(c.hch):
                    pb = bpool.tile([c.HCW, c.BC], F32, tag="pb")
                    for ci, (d0, dn) in enumerate(c.dch):
                        nc.tensor.matmul(
                            out=pb[0:hn, :],
                            lhsT=w1b_T(ci, h0, hn, dn),
                            rhs=pool_rhs(0, ci),
                            start=(ci == 0), stop=(ci == c.nd - 1))
                    nc.scalar.activation(
                        out=bias_sb[0:hn, hi, :], in_=pb[0:hn, :],
                        func=mybir.ActivationFunctionType.Identity,
                        bias=bias_ap("b1", hi, hn), scale=1.0)
                run_group(0, tpool, epool, hpool, spool, srpool, hspool)
                run_group(1, tpool, epool, hpool, spool, srpool, hspool)
                pool_half(1, ppool)     # obj pools; only gate the MLP
            with tc.tile_pool(name="apool", bufs=1, space="PSUM") as apool, \
                 tc.tile_pool(name="wpool", bufs=2, space="PSUM") as wpool:
                attn_half(0, apool, wpool, wrpool)
                run_group(2, tpool, epool, hpool, spool, srpool, hspool)
                run_group(3, tpool, epool, hpool, spool, srpool, hspool)
                attn_half(1, apool, wpool, wrpool)

        # ---- transpose sattn to D-major chunks [dlow, chunk, b]
        nc.vector.tensor_copy(out=sattn_bf[:], in_=sattn[:])
        with tc.tile_pool(name="stpool", bufs=2, space="PSUM") as stpool:
            for ci, (d0, dn) in enumerate(c.dch):
                pst = stpool.tile([c.CW, c.BC], BF16, tag="pst")
                nc.tensor.transpose(out=pst[0:dn, :], in_=sattn_bf[:, d0:d0 + dn],
                                    identity=ident[0:c.BC, 0:c.BC])
                nc.vector.tensor_copy(out=satd[0:dn, ci, :], in_=pst[0:dn, :])

        # ---- output MLP (N=BC is small)
        with tc.tile_pool(name="mpool", bufs=2, space="PSUM") as mpool, \
             tc.tile_pool(name="m2pool", bufs=2, space="PSUM") as m2pool:
            nk = 3 * c.nd
            for hi, (h0, hn) in enumerate(c.hch):
                pm = mpool.tile([c.HCW, c.BC], F32, tag="pm")
                for blk in range(3):
                    for ci, (d0, dn) in enumerate(c.dch):
                        k = blk * c.nd + ci
                        if blk == 0:
                            rhs = satd[0:dn, ci, :]
                        else:
                            rhs = pool_rhs(blk - 1, ci)
                        nc.tensor.matmul(
                            out=pm[0:hn, :],
                            lhsT=mw1_T(k, h0, hn, dn),
                            rhs=rhs, start=(k == 0), stop=(k == nk - 1))
                nc.scalar.activation(
                    out=o1_sb[0:hn, hi, :], in_=pm[0:hn, :],
                    func=mybir.ActivationFunctionType.Relu,
                    bias=bias_ap("mb1", hi, hn), scale=1.0)
            for hi, (h0, hn) in enumerate(c.hch):
                pm2 = m2pool.tile([c.HCW, c.BC], F32, tag="pm2")
                for ki, (k0, kn) in enumerate(c.hch):
                    nc.tensor.matmul(
                        out=pm2[0:hn, :],
                        lhsT=mw2_T(ki, h0, hn, kn),
                        rhs=o1_sb[0:kn, ki, :],
                        start=(ki == 0), stop=(ki == c.nh - 1))
                nc.scalar.activation(
                    out=out_sb[0:hn, hi, :], in_=pm2[0:hn, :],
                    func=mybir.ActivationFunctionType.Relu,
                    bias=bias_ap("mb2", hi, hn), scale=1.0)
            for hi in range(c.nh):
                nc.sync.dma_start(out=out_d[hi], in_=out_sb[:, hi, :])

    nc.finalize()
    nc._gather_call_names = calls
    return nc


def find_queue_map(cfg: Cfg):
    """Assign each gather call the queue implied by its scheduled DMASW
    lane (lane rotates per scheduled SWDGE DMA; a lane's semaphore is
    locked to one queue).  Iterate to a fixed point."""
    import bass_rust
    nsem = bass_rust.NUM_SWDGE_GLOBAL_SEMS
    qm = {}
    for _ in range(4):
        nc = build_nc(cfg, queue_map=qm)
        order = []
        for fn in nc.m.functions:
            for blk in fn.blocks:
                for inst in blk.instructions:
                    if 'Gather' in type(inst).__name__:
                        order.append((inst.name, getattr(inst, 'queue_num', 0)))
        emit = nc._gather_call_names
        by_emit = sorted((nm for nm, _ in order), key=lambda s: int(s.split('-')[1]))
        new_qm = {}
        ok = True
        lane_q = {}
        for sched_i, (nm, q) in enumerate(order):
            emit_idx = by_emit.index(nm)
            lane = sched_i % nsem
            want = lane_q.setdefault(lane, lane % 4)
            new_qm[emit[emit_idx]] = want
            if q != want:
                ok = False
        qm = new_qm
        if ok:
            return qm
    return qm


# ------------------------------------------------------------------ host


def wrap16(idx, n):
    """int16 index list -> [128, n/16] wrapped + replicated per Q7 core."""
    return np.ascontiguousarray(
        np.tile(np.asarray(idx).astype(np.int16).reshape(n // 16, 16).T, (8, 1)))


def to_bf16(x):
    import ml_dtypes
    return np.asarray(x, dtype=np.float32).astype(ml_dtypes.bfloat16)


def host_prepare(cfg: Cfg, words, subj_pos, obj_pos, emb_table,
                 w1, b1, w2, b2, mw1, mb1, mw2, mb2):
    import ml_dtypes
    c = cfg
    words = np.asarray(words).astype(np.int64)
    subj_pos = np.asarray(subj_pos)
    obj_pos = np.asarray(obj_pos)

    # sentinel rows at physical 0 and VP-1; word w -> row w + 1
    table = np.zeros((c.VP, c.E), ml_dtypes.bfloat16)
    table[1:1 + c.V, :c.D] = to_bf16(emb_table)
    table[0, :] = ml_dtypes.bfloat16(-NEG_INF)
    table[c.VP - 1, :] = ml_dtypes.bfloat16(-NEG_INF)

    w1 = np.asarray(w1, dtype=np.float32)
    w1a, w1b = w1[:c.D], w1[c.D:2 * c.D]
    mw1 = np.asarray(mw1, dtype=np.float32)
    mw1e = np.concatenate([mw1[0:c.D] + mw1[c.D:2 * c.D],
                           mw1[2 * c.D:3 * c.D], mw1[3 * c.D:4 * c.D]], axis=0)
    mw2 = np.asarray(mw2, dtype=np.float32)
    w2v = np.asarray(w2, dtype=np.float32).reshape(c.H)

    wblob = np.zeros((128, c.WBLOB), np.float32)
    for ci, (d0, dn) in enumerate(c.dch):
        wblob[0:dn, c.OW1A + ci * c.H:c.OW1A + (ci + 1) * c.H] = w1a[d0:d0 + dn]
        wblob[0:dn, c.OW1B + ci * c.H:c.OW1B + (ci + 1) * c.H] = w1b[d0:d0 + dn]
        for blk in range(3):
            k = blk * c.nd + ci
            wblob[0:dn, c.OMW1 + k * c.H:c.OMW1 + (k + 1) * c.H] = \
                mw1e[blk * c.D + d0:blk * c.D + d0 + dn]
    for ki, (k0, kn) in enumerate(c.hch):
        wblob[0:kn, c.OMW2 + ki * c.H:c.OMW2 + (ki + 1) * c.H] = mw2[k0:k0 + kn]
        wblob[0:kn, c.OW2 + ki] = w2v[k0:k0 + kn]

    bblob = np.zeros((c.HCW, 3 * c.nh), np.float32)
    for hi, (h0, hn) in enumerate(c.hch):
        bblob[0:hn, hi] = np.asarray(b1, np.float32).reshape(-1)[h0:h0 + hn]
        bblob[0:hn, c.nh + hi] = np.asarray(mb1, np.float32).reshape(-1)[h0:h0 + hn]
        bblob[0:hn, 2 * c.nh + hi] = np.asarray(mb2, np.float32).reshape(-1)[h0:h0 + hn]

    shared = {"table": table, "wblob": to_bf16(wblob),
              "bblob": np.ascontiguousarray(bblob)}
    HALF = c.L // 2
    in_maps = []
    for core in range(c.NCORES):
        b0 = core * c.BC
        lo_list, hi_list = [], []
        # pool slot arrays, ordered q = m*BC + b
        pl = np.zeros((2 * c.BC, c.PSLOT), np.int64)          # sentinel row 0
        ph = np.full((2 * c.BC, c.PSLOT), 32767, np.int64)    # sentinel VP-1
        for b in range(c.BC):
            w = words[b0 + b]
            order = np.argsort(w, kind="stable")
            ws = w[order] + 1                       # physical rows
            if ws[HALF - 1] >= 32768 or ws[HALF] < c.HB2:
                raise RuntimeError(
                    f"batch {b0 + b}: vocab split infeasible "
                    f"(lo_max={ws[HALF - 1]}, hi_min={ws[HALF]})")
            lo_list.append(ws[:HALF])
            hi_list.append(ws[HALF:] - c.HB2)
            for m, pos in ((0, subj_pos), (1, obj_pos)):
                kept = w[np.asarray(pos[b0 + b]) == 0] + 1    # physical rows
                lo = [int(x) for x in kept[kept < c.HB2]]
                hi = [int(x) for x in kept[kept > 32767]]
                for x in kept[(kept >= c.HB2) & (kept <= 32767)]:
                    (lo if len(lo) < c.PSLOT else hi).append(int(x))
                if len(lo) > c.PSLOT or len(hi) > c.PSLOT:
                    raise RuntimeError(f"pool slot overflow b={b0+b} m={m}: "
                                       f"{len(lo)}/{len(hi)}")
                # pad by cycling real entries (max-neutral); long runs of a
                # repeated sentinel index hit a slow path in the gather prep
                q = m * c.BC + b
                if lo:
                    pl[q] = [lo[i % len(lo)] for i in range(c.PSLOT)]
                if hi:
                    ph[q] = [hi[i % len(hi)] - c.HB2 for i in range(c.PSLOT)]
        iblob = np.concatenate([
            wrap16(np.concatenate(lo_list), c.T // 2),
            wrap16(np.concatenate(hi_list), c.T // 2),
            wrap16(pl.reshape(-1), c.NPOOL),
            wrap16(ph.reshape(-1), c.NPOOL)], axis=1)
        in_maps.append({"iblob": np.ascontiguousarray(iblob), **shared})
    return in_maps


def assemble_output(cfg: Cfg, results):
    c = cfg
    outs = []
    for core in range(c.NCORES):
        o = results[core]["out"]                      # [nh, HCW, BC]
        outs.append(o.reshape(c.H, c.BC).T)           # [BC, H]
    return np.ascontiguousarray(np.concatenate(outs, axis=0))


_CACHE = {}


def run(inputs, trace=False, **kw):
    from concourse.bass_utils import run_bass_kernel_spmd

    cfg = Cfg()
    in_maps = host_prepare(cfg, **{k: inputs[k] for k in (
        "words", "subj_pos", "obj_pos", "emb_table", "w1", "b1", "w2", "b2",
        "mw1", "mb1", "mw2", "mb2")})
    if "nc" not in _CACHE:
        qm = find_queue_map(cfg)
        _CACHE["nc"] = build_nc(cfg, queue_map=qm)
    nc = _CACHE["nc"]
    res = run_bass_kernel_spmd(nc, in_maps, core_ids=list(range(cfg.NCORES)),
                               trace=trace, **kw)
    return assemble_output(cfg, res.results), res


def kernel(**inputs) -> np.ndarray:
    return run(inputs)[0]
